# revision 23
# baseline (speedup 1.0000x reference)
"""ARAP loss kernel for Trainium2 (8 NeuronCores, Bass/Tile).

Mathematical reformulation (exact):
  reference loss = sum_n sum_k w (d - R_n r)^2  with R_n from SVD of
  S_n = sum_k (w r)_k d_k^T, R = V U^T. Since tr(R S) = sum of singular
  values (nuclear norm),
      loss = E1 - 2 * sum_n nuc(S_n),   E1 = sum_{n,k} w (|d|^2 + |r|^2).

Structure exploited (verified at runtime against elem_idx):
  * The mesh is the deterministic 512x512 grid of reference.py: the
    gather V[elem_idx] is a fixed stencil.
  * Each face's 3 edges are replicated to its 3 vertices with identical
    weights => per-vertex element lists collapse 3x to face-major form:
    S_n = sum_{f ni n} M_f with one shared 3x3 M_f per face, and
    E1 = 3 * sum_f e_f.
  * Triangle closure (r2 = -(r0+r1), d2 = -(d0+d1)) collapses the three
    outer products per face to two:  M_f = g0 d0^T + g1 d1^T  with
    g0 = (w0+w2) r0 + w2 r1,  g1 = (w1+w2) r1 + w2 r0.

Device layout (v2): partition p = column j div 4, jsub s = j mod 4, so
both stencil shifts are free-dim ops except the s=0 column, which takes
one small PE shift-matmul. Face-matrix middle layer runs in bf16
(validated: final rel err ~2.7e-3 vs 2e-2 budget); eigen chain in fp32.
Per-core work: outer products (DVE bf16) -> X/T/W stencil adds -> S ->
A = S^T S (bf16) -> closed-form eigenvalues (trig, fp32) -> nuc; e_d
via one fused tensor_tensor_reduce. Host: loss = 3*(e_d + e_r) - 2*nuc.
"""

import numpy as np
import ml_dtypes

import concourse.bacc as bacc
import concourse.bass as bass
import concourse.mybir as mybir
import concourse.tile as tile
from concourse.bass_utils import run_bass_kernel_spmd

F32 = mybir.dt.float32
BF16 = mybir.dt.bfloat16
AF = mybir.ActivationFunctionType
OP = mybir.AluOpType
NPBF = ml_dtypes.bfloat16

GRID = 512
CORES = 8


# ---------------------------------------------------------------------------
# host-side index structure (deterministic for the fixed grid)
# ---------------------------------------------------------------------------

def _grid_faces(n):
    idx = np.arange(n * n).reshape(n, n)
    v00 = idx[:-1, :-1].ravel(); v01 = idx[:-1, 1:].ravel()
    v10 = idx[1:, :-1].ravel(); v11 = idx[1:, 1:].ravel()
    F = np.concatenate(
        [np.stack([v00, v10, v11], 1), np.stack([v00, v11, v01], 1)], 0)
    return F


def _elem_maps(n):
    """(verts_s, pos, inv_order) of the reference element-list construction."""
    F = _grid_faces(n)
    verts = np.tile(F, (1, 3)).ravel()
    order = np.argsort(verts, kind='stable')
    verts_s = verts[order]
    counts = np.bincount(verts, minlength=n * n)
    starts = np.cumsum(counts) - counts
    pos = np.arange(verts.size) - np.repeat(starts, counts)
    inv = np.empty_like(order)
    inv[order] = np.arange(order.size)
    return F, verts_s, pos, inv


def _structure_ok(elem_idx, n):
    F, verts_s, pos, _ = _elem_maps(n)
    K = elem_idx.shape[1]
    es = np.repeat(F[:, [0, 1, 2]], 3, axis=1).ravel()
    et = np.repeat(F[:, [1, 2, 0]], 3, axis=1).ravel()
    rec = np.zeros((n * n, K, 2), dtype=elem_idx.dtype)
    order = np.argsort(np.tile(F, (1, 3)).ravel(), kind='stable')
    rec[verts_s, pos, 0] = es[order]
    rec[verts_s, pos, 1] = et[order]
    return np.array_equal(rec, np.asarray(elem_idx))


def _reference_fallback(V, elem_rest, elem_weights, elem_idx):
    """Exact numpy replica of the reference for unexpected inputs."""
    d = V[elem_idx[:, :, 1]] - V[elem_idx[:, :, 0]]
    w = elem_weights[:, :, None]
    S = np.einsum('nki,nkj->nij', elem_rest * w, d)
    U, _, Vt = np.linalg.svd(S)
    R = np.einsum('nji,nkj->nik', Vt, U)
    rest_rot = np.einsum('nij,nkj->nki', R, elem_rest)
    diff = d - rest_rot
    return np.asarray(np.sum(diff ** 2 * w), dtype=np.float32)


# ---------------------------------------------------------------------------
# host-side data prep
# ---------------------------------------------------------------------------

def _host_prep(V, elem_rest, elem_weights, grid=GRID, cores=CORES):
    n = grid
    ncell = n - 1
    rpc = n // cores          # vertex rows per core
    ci = rpc + 1              # cell rows per core incl. halo
    fhalf = ncell * ncell

    _, verts_s, pos, inv = _elem_maps(n)
    w9 = elem_weights[verts_s, pos][inv].reshape(-1, 9)
    r9 = elem_rest[verts_s, pos][inv].reshape(-1, 9, 3)
    wF = np.ascontiguousarray(w9[:, ::3])            # [Fn, 3]
    rF = np.ascontiguousarray(r9[:, ::3])            # [Fn, 3, 3]

    w0, w1, w2 = wF[:, 0], wF[:, 1], wF[:, 2]
    r0, r1, r2 = rF[:, 0], rF[:, 1], rF[:, 2]
    g0 = (w0 + w2)[:, None] * r0 + w2[:, None] * r1  # [Fn, 3]
    g1 = (w1 + w2)[:, None] * r1 + w2[:, None] * r0
    a = w0 + w2
    b = w1 + w2
    c2 = 2.0 * w2
    e_r_total = float(
        (w0.astype(np.float64) * (r0.astype(np.float64) ** 2).sum(1)
         + w1.astype(np.float64) * (r1.astype(np.float64) ** 2).sum(1)
         + w2.astype(np.float64) * (r2.astype(np.float64) ** 2).sum(1)).sum())

    def grd(x):  # [Fn/2, ...] lower/upper face grid [ncell, ncell, ...]
        return x.reshape(ncell, ncell, *x.shape[1:])

    # global per-cell feature grid: [cellrow + 1, jc, 30]
    # comps 0:12 = g vectors; 12:30 = e_d weights expanded per coordinate
    # (matching the d^2/cross term layout, so the weighted reduce needs no
    # broadcast access pattern)
    q = np.zeros((n + 1, n, 30), np.float32)
    rows = slice(1, ncell + 1)
    cols = slice(0, ncell)
    q[rows, cols, 0:3] = grd(g0[:fhalf])
    q[rows, cols, 3:6] = grd(g1[:fhalf])
    q[rows, cols, 6:9] = grd(g0[fhalf:])
    q[rows, cols, 9:12] = grd(g1[fhalf:])
    for k in range(3):
        q[rows, cols, 12 + k] = grd(a[:fhalf])
        q[rows, cols, 15 + k] = grd(b[:fhalf])
        q[rows, cols, 18 + k] = grd(a[fhalf:])
        q[rows, cols, 21 + k] = grd(b[fhalf:])
        q[rows, cols, 24 + k] = grd(c2[:fhalf])
        q[rows, cols, 27 + k] = grd(c2[fhalf:])

    vglob = np.zeros((n + 2, n, 3), np.float32)
    vglob[1:n + 1] = V.reshape(n, n, 3)

    g_maps = []
    v_maps = []
    for c in range(cores):
        gc = q[c * rpc: c * rpc + ci]                    # [ci, n, 18]
        g_maps.append(np.ascontiguousarray(
            gc.transpose(1, 2, 0)).astype(NPBF))         # [n,18,ci] bf16
        vc = vglob[c * rpc: c * rpc + ci + 1]            # [ci+1, n, 3]
        vi = np.zeros((n + 1, 3, ci + 1), np.float32)
        vi[:n] = vc.transpose(1, 2, 0)                   # [n, 3, ci+1]
        v_maps.append(vi)

    return g_maps, v_maps, e_r_total


def _shift_mat(pj):
    # out[m] = rhs[m-1] (out[0] = 0): lhs[p, p+1] = 1
    m = np.zeros((pj, 1, pj), np.float32)
    m[np.arange(pj - 1), 0, np.arange(1, pj)] = 1.0
    return m.astype(NPBF)


# ---------------------------------------------------------------------------
# device program
# ---------------------------------------------------------------------------

def build_bass(grid=GRID, cores=CORES):
    n = grid
    rpc = n // cores
    ci = rpc + 1              # 65
    vi = rpc + 2              # 66
    pj = 128
    sj = n // pj              # 4 jsub columns per partition

    nc = bacc.Bacc("TRN2", target_bir_lowering=False, debug=False,
                   enable_asserts=False)
    v_in = nc.dram_tensor("vtx", [n + 1, 3, vi], F32, kind="ExternalInput")
    g_in = nc.dram_tensor("gfc", [n, 30, ci], BF16, kind="ExternalInput")
    m_in = nc.dram_tensor("mats", [pj, 1, pj], BF16, kind="ExternalInput")
    out = nc.dram_tensor("out", [pj, 8], F32, kind="ExternalOutput")

    with tile.TileContext(nc) as tc:
        _emit(tc, v_in.ap(), g_in.ap(), m_in.ap(), out.ap(),
              n, rpc, ci, vi, pj, sj)
    nc.compile()
    return nc


def _emit(tc, v_in, g_in, m_in, out, n, rpc, ci, vi, pj, sj):
    from contextlib import ExitStack
    nc = tc.nc
    ni = sj * rpc             # 256 vertices per partition
    ctx = ExitStack()
    with ctx:
        sg = ctx.enter_context(tc.tile_pool(name="sg", bufs=1))
        psum = ctx.enter_context(tc.tile_pool(name="psum", bufs=1, space="PSUM"))

        def st(shape, dtype, tag):
            return sg.tile([pj] + shape, dtype, name=tag, tag=tag)

        # ---- inputs (g comps and e_d weights split so the outer products
        # can start before the weight DMA lands) ----------------------
        mats = st([1, pj], BF16, "mats")
        nc.sync.dma_start(out=mats, in_=m_in)
        vt = st([sj, 3, vi], F32, "vt")
        nc.sync.dma_start(
            out=vt, in_=v_in[0:n].rearrange('(p s) c i -> p s c i', s=sj))
        vs = st([sj, 3, vi], F32, "vs")
        nc.scalar.dma_start(
            out=vs, in_=v_in[1:n + 1].rearrange('(p s) c i -> p s c i', s=sj))
        gr = g_in.rearrange('(p s) c i -> p s c i', s=sj)
        gt = st([sj, 12, ci], BF16, "gt")
        nc.sync.dma_start(out=gt, in_=gr[:, :, 0:12, :])
        wt = st([sj, 18, ci], BF16, "wt")
        nc.scalar.dma_start(out=wt, in_=gr[:, :, 12:30, :])

        # constant bias tiles for ACT ops
        bias0 = st([1], F32, "bias0")
        nc.vector.memset(bias0, 0.0)
        sinb = st([3], F32, "sinb")
        for k, bv in enumerate((2 * np.pi / 3, 0.0, -np.pi / 3)):
            nc.gpsimd.memset(sinb[:, k:k + 1], float(bv))
        outp = st([8], F32, "outp")
        nc.gpsimd.memset(outp, 0.0)

        # ---- d vectors [pj, sj, 12, ci] bf16 ------------------------
        d = st([sj, 12, ci], BF16, "d")
        v0 = vt[:, :, :, 0:ci]
        v1 = vt[:, :, :, 1:ci + 1]
        s0 = vs[:, :, :, 0:ci]
        s1 = vs[:, :, :, 1:ci + 1]
        nc.vector.tensor_sub(d[:, :, 0:3, :], v1, v0)    # dL0
        nc.vector.tensor_sub(d[:, :, 3:6, :], s1, v1)    # dL1
        nc.vector.tensor_add(d[:, :, 6:9, :], d[:, :, 0:3, :],
                             d[:, :, 3:6, :])             # dU0 = dL0+dL1
        nc.vector.tensor_sub(d[:, :, 9:12, :], s0, s1)   # dU1

        # ---- e_d terms: squares on ACT, cross products on DVE -------
        edt = st([sj, 18, ci], BF16, "edt")
        nc.scalar.activation(edt[:, :, 0:12, :], d, AF.Square, bias=bias0)
        nc.gpsimd.tensor_tensor(edt[:, :, 12:15, :], d[:, :, 0:3, :],
                                d[:, :, 3:6, :], OP.mult)
        nc.gpsimd.tensor_tensor(edt[:, :, 15:18, :], d[:, :, 6:9, :],
                                d[:, :, 9:12, :], OP.mult)
        # weight-multiply on DVE, then reduce on ACT (Identity + accum)
        eds = st([sj, 18, rpc], BF16, "eds")
        edump = st([sj, 18, rpc], BF16, "edump")
        nc.gpsimd.tensor_tensor(eds, edt[:, :, :, 1:ci], wt[:, :, :, 1:ci],
                                OP.mult)
        nc.scalar.activation(edump, eds, AF.Identity, bias=bias0,
                             accum_out=outp[:, 2:3])

        # ---- face matrices ML, MU [pj, 9, sj, ci] bf16 (comp-major) -
        ml = st([9, sj, ci], BF16, "ml")
        mu = st([9, sj, ci], BF16, "mu")
        tmpo = st([9, sj, ci], BF16, "tmpo")

        def outer(dst, gc0, dc0):
            # dst[3a+b, s, i] = g[a, s, i] * d[b, s, i]  (TT is limited to
            # 3 free dims, so one instruction per a-component)
            din = d[:, :, dc0:dc0 + 3, :].rearrange('p s b i -> p b s i')
            for a in range(3):
                gin = gt[:, :, gc0 + a, :]\
                    .rearrange('p s i -> p () s i')\
                    .broadcast_to([pj, 3, sj, ci])
                nc.vector.tensor_tensor(
                    dst[:, 3 * a:3 * a + 3, :, :], gin, din, OP.mult)

        outer(tmpo, 0, 0)       # g0L x dL0
        outer(ml, 3, 3)         # g1L x dL1
        nc.vector.tensor_add(ml, ml, tmpo)
        outer(tmpo, 6, 6)       # g0U x dU0
        outer(mu, 9, 9)         # g1U x dU1
        nc.vector.tensor_add(mu, mu, tmpo)

        # ---- stencil: X = MU(i)+ML(i-1); T = ML(i)+X; W = X+MU(i-1) -
        xb = st([9, sj, rpc], BF16, "xb")
        tb = st([9, sj, rpc], BF16, "tb")
        wb = st([9, sj, rpc], BF16, "wb")
        nc.vector.tensor_add(xb, mu[:, :, :, 1:ci], ml[:, :, :, 0:rpc])
        nc.vector.tensor_add(tb, ml[:, :, :, 1:ci], xb)
        nc.vector.tensor_add(wb, xb, mu[:, :, :, 0:rpc])

        # ---- column shift: s0psum[q*rpc+i] = W[p-1, q, 3, i] on PE --
        s0ps = psum.tile([pj, 9 * rpc], F32, name="s0ps", tag="s0ps")
        bank = 512
        nq0 = bank // rpc       # 8 comps in bank 0
        for lo, cnt in ((0, nq0), (nq0, 9 - nq0)):
            o = s0ps[:, lo * rpc:(lo + cnt) * rpc]\
                .rearrange('p (q i) -> p q i', q=cnt)
            nc.tensor.matmul(o, mats[:, 0, :], wb[:, lo:lo + cnt, sj - 1, :],
                             start=True, stop=True)

        # ---- S [pj, 9, sj, rpc] bf16 --------------------------------
        sS = st([9, sj, rpc], BF16, "sS")
        nc.vector.tensor_add(sS[:, :, 1:sj, :], tb[:, :, 1:sj, :],
                             wb[:, :, 0:sj - 1, :])
        nc.vector.tensor_add(
            sS[:, :, 0, :], tb[:, :, 0, :],
            s0ps.rearrange('p (q i) -> p q i', q=9))

        # ---- A = S^T S (6 comps) bf16 -------------------------------
        sq = st([9, ni], BF16, "sq")
        sf = sS.rearrange('p q s i -> p q (s i)')
        nc.scalar.activation(sq, sf, AF.Square, bias=bias0)
        dumm = st([1], F32, "dumm")
        nc.scalar.activation(dumm, bias0, AF.Sqrt, bias=bias0)
        a_all = st([6, ni], BF16, "a_all")
        nc.vector.tensor_add(a_all[:, 0:3, :], sq[:, 0:3, :], sq[:, 3:6, :])
        nc.vector.tensor_add(a_all[:, 0:3, :], a_all[:, 0:3, :], sq[:, 6:9, :])
        paw = st([9, ni], BF16, "paw")
        s3 = sf.rearrange('p (a c) x -> p a c x', a=3)
        nc.vector.tensor_tensor(
            paw[:, 0:6, :].rearrange('p (a k) x -> p a k x', a=3),
            s3[:, :, 0, :].rearrange('p a x -> p a () x')
              .broadcast_to([pj, 3, 2, ni]),
            s3[:, :, 1:3, :], OP.mult)
        nc.vector.tensor_tensor(
            paw[:, 6:9, :], s3[:, :, 1, :], s3[:, :, 2, :], OP.mult)
        nc.gpsimd.tensor_add(
            a_all[:, 3:5, :],
            paw[:, 0:2, :], paw[:, 2:4, :])
        nc.vector.tensor_add(a_all[:, 3:5, :], a_all[:, 3:5, :],
                             paw[:, 4:6, :])
        nc.gpsimd.tensor_add(a_all[:, 5, :], paw[:, 6, :], paw[:, 7, :])
        nc.vector.tensor_add(a_all[:, 5, :], a_all[:, 5, :], paw[:, 8, :])

        # ---- phase 2: eigenvalues + nuclear norm, 2 pipelined chunks.
        # Main chain on DVE; independent side products on Pool; table
        # functions (and foldable scalar factors) on ACT. Chunks are
        # emitted interleaved so both chains advance together.
        nch = 2
        fch = ni // nch
        C = range(nch)

        def t2(tag, c, comps=None):
            shape = [fch] if comps is None else [comps, fch]
            return sg.tile([pj] + shape, F32, name=f"{tag}{c}", tag=f"{tag}{c}")

        def bc3(x):
            return x.rearrange('p (k f) -> p k f', k=1).broadcast_to([pj, 3, fch])

        A_ = [a_all[:, :, slice(c * fch, (c + 1) * fch)] for c in C]
        q3 = [t2("q3", c) for c in C]
        bd = [t2("bd", c, 3) for c in C]
        sq6 = [t2("sq6", c, 6) for c in C]
        sd = [t2("sd", c) for c in C]
        so = [t2("so", c) for c in C]
        p2 = [t2("p2", c) for c in C]
        x1 = [t2("x1", c) for c in C]
        x2 = [t2("x2", c) for c in C]
        x3 = [t2("x3", c) for c in C]
        x4 = [t2("x4", c) for c in C]
        x5 = [t2("x5", c) for c in C]
        det = [t2("det", c) for c in C]
        tsq = [t2("tsq", c) for c in C]
        u = [t2("u", c) for c in C]
        dt2 = [t2("dt2", c) for c in C]
        ru = [t2("ru", c) for c in C]
        rs = [t2("rs", c) for c in C]
        tp = [t2("tp", c) for c in C]
        arg = [t2("arg", c) for c in C]
        at = [t2("at", c) for c in C]
        cs = [t2("cs", c, 3) for c in C]
        lam = [t2("lam", c, 3) for c in C]
        sgr = [t2("sgr", c, 3) for c in C]
        n1 = [t2("n1", c) for c in C]

        for c in C:
            nc.gpsimd.tensor_add(q3[c], A_[c][:, 0, :], A_[c][:, 1, :])
        for c in C:
            nc.gpsimd.tensor_add(q3[c], q3[c], A_[c][:, 2, :])
        for c in C:
            # bd = A_diag - q3/3 in one fused op
            nc.vector.scalar_tensor_tensor(
                bd[c], bc3(q3[c]), -1.0 / 3.0, A_[c][:, 0:3, :],
                OP.mult, OP.add)
        for c in C:
            nc.scalar.activation(sq6[c][:, 3:6, :], A_[c][:, 3:6, :],
                                 AF.Square, bias=bias0)
        for c in C:
            nc.scalar.activation(sq6[c][:, 0:3, :], bd[c], AF.Square,
                                 bias=bias0)
        for c in C:
            # det = b0*b1*b2 - b0*o12^2 - b2*o01^2 - b1*o02^2 + 2*o01*o02*o12
            b0, b1, b2 = bd[c][:, 0, :], bd[c][:, 1, :], bd[c][:, 2, :]
            o01, o02, o12 = A_[c][:, 3, :], A_[c][:, 4, :], A_[c][:, 5, :]
            nc.vector.tensor_mul(x1[c], b0, b1)
            nc.gpsimd.tensor_mul(x2[c], b0, sq6[c][:, 5, :])
            nc.gpsimd.tensor_mul(x3[c], b2, sq6[c][:, 3, :])
            nc.gpsimd.tensor_mul(x4[c], b1, sq6[c][:, 4, :])
            nc.vector.tensor_mul(x5[c], o01, o02)
            nc.vector.tensor_add(sd[c], sq6[c][:, 0, :], sq6[c][:, 1, :])
            nc.vector.tensor_add(so[c], sq6[c][:, 3, :], sq6[c][:, 4, :])
            nc.vector.tensor_mul(x1[c], x1[c], b2)
            nc.gpsimd.tensor_add(x2[c], x2[c], x3[c])
            nc.vector.tensor_mul(x5[c], x5[c], o12)
            nc.vector.tensor_add(sd[c], sd[c], sq6[c][:, 2, :])
            nc.vector.tensor_add(so[c], so[c], sq6[c][:, 5, :])
            nc.gpsimd.tensor_add(x2[c], x2[c], x4[c])
        for c in C:
            nc.vector.scalar_tensor_tensor(p2[c], so[c], 2.0, sd[c],
                                           OP.mult, OP.add)
            nc.vector.scalar_tensor_tensor(det[c], x5[c], 2.0, x1[c],
                                           OP.mult, OP.add)
        for c in C:
            nc.vector.tensor_sub(det[c], det[c], x2[c])
            nc.vector.tensor_mul(tsq[c], p2[c], p2[c])
            # tp = 2*sqrt(p2/6) = sqrt(p2 * 2/3), folded into one ACT op
            nc.scalar.activation(tp[c], p2[c], AF.Sqrt, bias=bias0,
                                 scale=2.0 / 3.0)
        for c in C:
            nc.vector.scalar_tensor_tensor(u[c], p2[c], 1.0 / 54.0, tsq[c],
                                           OP.mult, OP.mult)
            nc.scalar.activation(dt2[c], det[c], AF.Square, bias=bias0)
        for c in C:
            nc.vector.tensor_sub(u[c], u[c], dt2[c])
        for c in C:
            nc.vector.tensor_scalar_max(u[c], u[c], 1e-30)
        for c in C:
            nc.vector.reciprocal_approx_fast(ru[c], u[c])
        for c in C:
            nc.scalar.activation(rs[c], ru[c], AF.Sqrt, bias=bias0)
        for c in C:
            nc.vector.tensor_mul(arg[c], det[c], rs[c])
        dums = st([1], F32, "dums")
        nc.scalar.activation(dums, arg[0][:, 0:1], AF.Sin, bias=bias0,
                             scale=0.0)
        for c in C:
            # the TRN2 Arctan table is accurate over the full input range
            # (verified on hardware), so no range reduction is needed
            nc.scalar.activation(at[c], arg[c], AF.Arctan, bias=bias0)
        # cs_k = cos(theta + phi_k), theta = pi/6 - at/3, via Sin
        for c in C:
            for k, sc in enumerate((-1.0 / 3.0, -1.0 / 3.0, 1.0 / 3.0)):
                nc.scalar.activation(cs[c][:, k, :], at[c], AF.Sin,
                                     bias=sinb[:, k:k + 1], scale=sc)
        for c in C:
            nc.vector.tensor_tensor(lam[c], cs[c], bc3(tp[c]), OP.mult)
        for c in C:
            # lam += q3/3, fused
            nc.vector.scalar_tensor_tensor(lam[c], bc3(q3[c]), 1.0 / 3.0,
                                           lam[c], OP.mult, OP.add)
        for c in C:
            nc.vector.tensor_scalar_max(lam[c], lam[c], 0.0)
        for c in C:
            nc.scalar.activation(sgr[c], lam[c], AF.Sqrt, bias=bias0)
        for c in C:
            nc.vector.tensor_add(n1[c], sgr[c][:, 0, :], sgr[c][:, 1, :])
        for c in C:
            nc.vector.tensor_add(n1[c], n1[c], sgr[c][:, 2, :])
        for c in C:
            nc.vector.tensor_reduce(outp[:, c:c + 1], n1[c],
                                    mybir.AxisListType.X, OP.add)

        nc.sync.dma_start(out=out, in_=outp)


# ---------------------------------------------------------------------------
# entry point
# ---------------------------------------------------------------------------

_NC_CACHE = {}


def _get_nc(grid=GRID, cores=CORES):
    key = (grid, cores)
    if key not in _NC_CACHE:
        _NC_CACHE[key] = build_bass(grid, cores)
    return _NC_CACHE[key]


def run_device(g_maps, v_maps, grid=GRID, cores=CORES, trace=False):
    nc = _get_nc(grid, cores)
    mats = _shift_mat(128)
    in_maps = [{"vtx": v_maps[c], "gfc": g_maps[c], "mats": mats}
               for c in range(cores)]
    res = run_bass_kernel_spmd(nc, in_maps, core_ids=list(range(cores)),
                               trace=trace)
    return res


def kernel(V_deformed, elem_rest, elem_weights, elem_idx):
    V = np.asarray(V_deformed, np.float32)
    er = np.asarray(elem_rest, np.float32)
    ew = np.asarray(elem_weights, np.float32)
    ei = np.asarray(elem_idx)
    n = GRID
    assert V.shape == (n * n, 3)

    if not _structure_ok(ei, n):
        return _reference_fallback(V, er, ew, ei)

    g_maps, v_maps, e_r_total = _host_prep(V, er, ew, n, CORES)
    res = run_device(g_maps, v_maps, n, CORES)
    nuc_sum = 0.0
    e_sum = 0.0
    for r in res.results:
        o = r["out"].astype(np.float64)
        nuc_sum += o[:, 0].sum() + o[:, 1].sum()
        e_sum += o[:, 2].sum()
    loss = 3.0 * (e_sum + e_r_total) - 2.0 * nuc_sum
    return np.asarray(loss, dtype=np.float32)
